# revision 13
# baseline (speedup 1.0000x reference)
"""MQA kernel for Trainium2 (8 NeuronCores, SPMD via bass/Tile).

Problem: nn_MultiQueryAttention (B=2, T=2048, HID=2048, H=16, D=128).

Key algebraic simplification: the reference's apply_rope treats q's layout
as (B,T,H,D) while q is actually (B,H,T,D), so the "position" axis is the
head index -> per-head rotation R_h acting on the D axis only, independent
of sequence position. R_h is folded into Wq on the host. k's rope at pos=0
is a pure channel permutation, folded into Wk. The score scale 1/sqrt(D)
is folded into Wq as well. What remains on-device is a plain causal MQA.

Sharding (uniform SPMD program, per-core data differs):
  core c -> batch c//4, heads (c%4)*4..(c%4)*4+3, full T.
  Each core: Q^T/K^T/V projections, causal softmax attention for its 4
  heads, and a partial out-projection (its heads' rows of Wo^T). The 4
  partials per batch are summed on the host.

Single-pass pipeline over 4 t-blocks of 512: project Q/K/V for the block,
run the 4 causal attention rows that became computable, then the block's
out-projection as one dense matmul sweep. hs-tile DMA for block b+1
overlaps block b (prefetch depth 16 ~ all DMA engines); DMA issue is
spread over the gpsimd (hs), scalar (Wq) and sync (Wk/Wv/Wo, outputs)
queues since each issue costs ~0.6us of queue time.

Precision: the PE streams 1 column/cycle for 2-byte dtypes with fast
weight loads; the Q/K path (hs, Wq, Wk/Wv, Q^T, K^T) runs in fp16 whose
11-bit mantissa keeps softmax scores accurate; probs/V/out-proj run in
bf16 (exp(40) range); all matmuls accumulate fp32 in PSUM.

Attention processes all 4 heads per matmul; scores for two key tiles land
in one 2-bank PSUM tile so a single 1024-wide ACT exp covers both (exp
would otherwise out-pace the PE). Softmax denominators come from
column-tiled M=1 ones-matmuls: 4 strips at tile_position (0,32j) run
concurrently in the PE array, so each exp tile costs ~1/4 matmul instead
of a full 512-column pass; a select-column matmul recombines the strips.
Each row's tail (reciprocal, rank-1 1/den broadcast matmul, normalize
into the block's at tile) is emitted inside the NEXT row so the PE never
waits on the DVE chain.
"""

import numpy as np
import ml_dtypes
from contextlib import ExitStack

import concourse.bass as bass
import concourse.tile as tile
from concourse import bacc, mybir
from concourse.bass_utils import run_bass_kernel_spmd
from concourse.masks import make_identity

F32 = mybir.dt.float32
FP16 = mybir.dt.float16
BF16 = mybir.dt.bfloat16
EXP = mybir.ActivationFunctionType.Exp

B, T, HID, H, D = 2, 2048, 2048, 16, 128
NCORES = 8
CPB = 4              # cores per batch
HPC = H // CPB       # 4 heads per core
HD_PC = HPC * D      # 512 output dims per core
P = 128
KT = T // P          # 16 key tiles
NK = HID // P        # 16 contraction tiles for projections
NBLK = 4             # t blocks of 512
TPB = 4              # query tiles per block


def _rope_fold():
    """Per-head rotation matrices R_h (128x128) from the reference's quirky rope."""
    half = D // 2
    theta = 1.0 / (10000.0 ** (np.arange(0, half, 2, dtype=np.float64) / half))
    mats = []
    for h in range(H):
        R = np.zeros((D, D), dtype=np.float64)
        c = np.cos(h * theta)
        s = np.sin(h * theta)
        for j in range(32):
            R[j, 2 * j] = c[j]
            R[j, 2 * j + 1] = -s[j]
            R[32 + j, 2 * j] = s[j]
            R[32 + j, 2 * j + 1] = c[j]
            R[64 + j, 64 + 2 * j] = c[j]
            R[64 + j, 64 + 2 * j + 1] = -s[j]
            R[96 + j, 64 + 2 * j] = s[j]
            R[96 + j, 64 + 2 * j + 1] = c[j]
        mats.append(R)
    return mats


def _build_program():
    nc = bacc.Bacc("TRN2", target_bir_lowering=False, debug=False,
                   enable_asserts=False, num_devices=NCORES)

    hsT = nc.dram_tensor("hsT", [HID, T], FP16, kind="ExternalInput").ap()
    wqT = nc.dram_tensor("wqT", [HID, HD_PC], FP16, kind="ExternalInput").ap()
    wkvT = nc.dram_tensor("wkvT", [HID, 2 * D], FP16, kind="ExternalInput").ap()
    woT = nc.dram_tensor("woT", [HD_PC, HID], BF16, kind="ExternalInput").ap()
    dmd = nc.dram_tensor("dmask", [P, P], BF16, kind="ExternalInput").ap()
    onbf = nc.dram_tensor("onbf", [P, P], BF16, kind="ExternalInput").ap()
    seld = nc.dram_tensor("seld", [P, P], BF16, kind="ExternalInput").ap()
    out = nc.dram_tensor("out", [T, HID], BF16, kind="ExternalOutput").ap()

    hsT_r = hsT.rearrange("(ko p) t -> ko p t", p=P)        # [16,128,2048]
    wqT_r = wqT.rearrange("(ko p) m -> p ko m", p=P)        # [128,16,512]
    wkvT_r = wkvT.rearrange("(ko p) d -> p ko d", p=P)      # [128,16,256]
    woT_r = woT.rearrange("(h p) n -> p h n", p=P)          # [128,4,2048]
    out_r = out.rearrange("(tt p) n -> tt p n", p=P)        # [16,128,2048]

    def mm(ps, lhsT, rhs, start, stop):
        nc.tensor.matmul(ps, lhsT=lhsT, rhs=rhs, start=start, stop=stop)

    with tile.TileContext(nc) as tc, ExitStack() as ctx:
        singles = ctx.enter_context(tc.tile_pool(name="singles", bufs=1))
        hpool = ctx.enter_context(tc.tile_pool(name="hst", bufs=16))
        epool = ctx.enter_context(tc.tile_pool(name="etile", bufs=6))
        spool = ctx.enter_context(tc.tile_pool(name="small", bufs=2))
        apool = ctx.enter_context(tc.tile_pool(name="att", bufs=2))
        opool = ctx.enter_context(tc.tile_pool(name="outt", bufs=4))

        ident = singles.tile([P, P], F32)
        make_identity(nc, ident)
        ident_bf = singles.tile([P, P], BF16)
        nc.vector.tensor_copy(ident_bf[:], ident[:])
        dmask = singles.tile([P, P], BF16)
        ones_bf = singles.tile([P, P], BF16)
        sel_bf = singles.tile([P, P], BF16)
        nc.sync.dma_start(out=dmask, in_=dmd)
        nc.sync.dma_start(out=ones_bf, in_=onbf)
        nc.sync.dma_start(out=sel_bf, in_=seld)

        # weight residents; per-k slices stream in with the first block's
        # data. Wo rides the sync queue (outputs only start later).
        wq_sb = singles.tile([P, NK, HD_PC], FP16)
        wkv_sb = singles.tile([P, NK, 2 * D], FP16)
        wo_sb = singles.tile([P, HPC, HID], BF16)

        # resident activations
        qt_sb = singles.tile([P, HPC, T], FP16)      # Q^T per head [d, t]
        kt_sb = singles.tile([P, T], FP16)           # K^T [d, s]
        v_sb = singles.tile([P, KT, D], BF16)        # V natural [s-tile, d]

        dmask_b = dmask[:, None, :].to_broadcast([P, HPC, P])
        prev_at_ref = [None]  # at tile of the previous block (sweep deferral)

        def tail_a(pp, st_):
            """First half of a row's softmax tail: recombine strips, 1/den."""
            den_ps, strips = st_["den_ps"], st_["strips"]
            if strips:  # recombine the 4 column-tiled strip rows
                dstr = spool.tile([P, HPC, P], BF16, tag="dstr", name="dstr")
                nc.vector.tensor_copy(dstr[:], den_ps[:])
                dt_ps = pp.tile([P, HPC, P], F32, tag="pp", name="dt_ps")
                mm(dt_ps[:1], sel_bf[:, :1], dstr[:], True, True)
                den_ap = dt_ps[:1]
            else:
                den_ap = den_ps[:1]
            recip = spool.tile([1, HPC, P], F32, tag="recip", name="recip")
            nc.vector.reciprocal_approx_fast(out=recip[:], in_=den_ap)
            recr = spool.tile([1, HPC, P], BF16, tag="recr", name="recr")
            nc.vector.tensor_copy(recr[:], recip[:])
            st_["recr"] = recr

        def tail_b(pp, st_):
            """Second half: broadcast 1/den across partitions, normalize O^T."""
            bc_ps = pp.tile([P, HPC, P], F32, tag="pp", name="bc_ps")
            mm(bc_ps[:], ones_bf[:1, :], st_["recr"][:], True, True)
            bc_sb = spool.tile([P, HPC, P], F32, tag="bc_sb", name="bc_sb")
            nc.vector.tensor_copy(bc_sb[:], bc_ps[:])
            nc.vector.tensor_mul(st_["at_blk"][:, :, st_["tt"], :],
                                 st_["ot_ps"][:], bc_sb[:])

        for blk in range(NBLK):
            tsl = slice(blk * 512, (blk + 1) * 512)
            # ---------------- projection for this 512-block ----------------
            with tc.tile_pool(name="ps1", bufs=1, space="PSUM") as ps1, \
                 tc.tile_pool(name="ps1t", bufs=2, space="PSUM") as ps1t:
                q_ps = [ps1.tile([P, 512], F32, tag=f"qps{h}", name=f"qps{h}")
                        for h in range(HPC)]
                k_ps = ps1.tile([P, 512], F32, tag="kps")
                v_ps = ps1.tile([P, 512], F32, tag="vps")
                for k in range(NK):
                    if blk == 0:  # stream weight slices just ahead of data
                        nc.sync.dma_start(out=wkv_sb[:, k, :], in_=wkvT_r[:, k, :])
                        nc.scalar.dma_start(out=wq_sb[:, k, :], in_=wqT_r[:, k, :])
                    hst = hpool.tile([P, 512], FP16)
                    nc.gpsimd.dma_start(out=hst, in_=hsT_r[k][:, tsl])
                    st, sp = (k == 0), (k == NK - 1)
                    for h in range(HPC):
                        mm(q_ps[h][:], wq_sb[:, k, h * D:(h + 1) * D], hst[:], st, sp)
                    mm(k_ps[:], wkv_sb[:, k, :D], hst[:], st, sp)
                    mm(v_ps[:], wkv_sb[:, k, D:], hst[:], st, sp)
                if blk == 0:  # out-proj weights: first needed at block 0's sweep
                    for h in range(HPC):
                        for jb in range(4):
                            nc.sync.dma_start(
                                out=wo_sb[:, h, jb * 512:(jb + 1) * 512],
                                in_=woT_r[:, h, jb * 512:(jb + 1) * 512])
                # V^T -> V natural via PE transpose (bf16); vt copy first so
                # the transposes start while the qt copies drain, and qt in
                # row-tile slices so row blk*4's scores start after 4 copies.
                vt_sb = spool.tile([P, 512], BF16, tag="vt")
                nc.vector.tensor_copy(vt_sb[:], v_ps[:])
                for tt in range(TPB):
                    for h in range(HPC):
                        qsl = slice(blk * 512 + tt * P, blk * 512 + (tt + 1) * P)
                        nc.vector.tensor_copy(qt_sb[:, h, qsl],
                                              q_ps[h][:, tt * P:(tt + 1) * P])
                nc.vector.tensor_copy(kt_sb[:, tsl], k_ps[:])
                for si in range(4):
                    pt = ps1t.tile([P, P], BF16, tag="tps")
                    nc.tensor.transpose(pt[:], vt_sb[:, si * P:(si + 1) * P],
                                        ident_bf[:])
                    nc.vector.tensor_copy(v_sb[:, blk * 4 + si, :], pt[:])

            # ---------------- attention rows of this block -----------------
            # The previous block's out-projection (4 matmuls + a copy per
            # item) interleaves between pairs as PE filler while the ACT exp
            # chain runs; its op groups share the pp PSUM slot with the
            # tails' dt/bc matmuls.
            prev_at = prev_at_ref[0]
            sweep_items = ([] if prev_at is None else
                           [(prev_at, tt, jb) for tt in range(TPB)
                            for jb in range(4)])
            sweep_oto = {}

            def emit_sweep(pp, n):
                for _ in range(n):
                    if not sweep_items:
                        return
                    at_prev, tt2, jb = sweep_items.pop(0)
                    tb2 = (blk - 1) * TPB + tt2
                    if tt2 not in sweep_oto:
                        sweep_oto[tt2] = opool.tile([P, 2048], BF16, tag="oto",
                                                    name="oto")
                    jsl = slice(jb * 512, (jb + 1) * 512)
                    op_ps = pp.tile([P, HPC, P], F32, tag="pp", name="op_ps")
                    for h in range(HPC):
                        mm(op_ps[:], at_prev[:, h, tt2, :], wo_sb[:, h, jsl],
                           h == 0, h == HPC - 1)
                    nc.vector.tensor_copy(sweep_oto[tt2][:, jsl], op_ps[:])
                    if jb == 3:
                        nc.sync.dma_start(out=out_r[tb2], in_=sweep_oto[tt2][:])

            at_blk = apool.tile([P, HPC, TPB, P], BF16, tag="atb", name="at_blk")
            with tc.tile_pool(name="ps2s", bufs=2, space="PSUM") as ps2s, \
                 tc.tile_pool(name="ps2o", bufs=2, space="PSUM") as ps2o, \
                 tc.tile_pool(name="ps2d", bufs=1, space="PSUM") as ps2d, \
                 tc.tile_pool(name="ps2b", bufs=1, space="PSUM") as ps2b:
                pend = None  # previous row awaiting its softmax tail
                for tt in range(TPB):
                    tb = blk * TPB + tt
                    strips = tb >= 3  # col-tiled den needs all 4 strip rows
                    qsl = slice(tb * P, (tb + 1) * P)
                    qrhs = qt_sb[:, :, qsl]              # [128, 4, 128]
                    ot_ps = ps2o.tile([P, HPC, P], F32, tag="ot")
                    den_ps = ps2d.tile([P, HPC, P], F32, tag="den")
                    pstrips = []  # (st, e_ap) awaiting a 4-wide strip flush
                    npairs = (tb + 2) // 2
                    for pi, st0 in enumerate(range(0, tb + 1, 2)):
                        npair = min(2, tb + 1 - st0)
                        s_ps = ps2s.tile([P, 2, HPC, P], F32, tag="sps")
                        for i in range(npair):
                            st = st0 + i
                            mm(s_ps[:, i], kt_sb[:, st * P:(st + 1) * P],
                               qrhs, True, True)
                        e_sb = epool.tile([P, 2, HPC, P], BF16, tag="etile")
                        if npair == 2:
                            nc.scalar.activation(e_sb[:], s_ps[:], EXP)
                        else:
                            nc.scalar.activation(e_sb[:, 0], s_ps[:, 0], EXP)
                        if st0 + npair - 1 == tb:  # diagonal tile: causal mask
                            nc.vector.tensor_mul(e_sb[:, npair - 1],
                                                 e_sb[:, npair - 1], dmask_b)
                        for i in range(npair):
                            st = st0 + i
                            if strips:
                                pstrips.append((st, e_sb[:, i]))
                            else:
                                mm(den_ps[:1], ones_bf[:, :1], e_sb[:, i],
                                   st == 0, st == tb)
                            mm(ot_ps[:], v_sb[:, st, :], e_sb[:, i],
                               st == 0, st == tb)
                        # den strips: emit back-to-back in groups of 4 so the
                        # M=1 matmuls run concurrently in distinct col groups
                        if len(pstrips) >= 4 or st0 + npair - 1 == tb:
                            for st, e_ap in pstrips:
                                j = st % 4
                                nc.tensor.matmul(
                                    den_ps[32 * j:32 * j + 1, :],
                                    lhsT=ones_bf[:, :1], rhs=e_ap,
                                    start=(st < 4), stop=(st + 4 > tb),
                                    tile_position=(0, 32 * j))
                            pstrips = []
                        if pend is not None:
                            if pi == 0:
                                tail_a(ps2b, pend)
                            if pi == min(1, npairs - 1):
                                tail_b(ps2b, pend)
                                pend = None
                        emit_sweep(ps2b, 2 if pi < 2 else 1)
                    pend = {"den_ps": den_ps, "strips": strips, "ot_ps": ot_ps,
                            "at_blk": at_blk, "tt": tt}
                tail_a(ps2b, pend)
                emit_sweep(ps2b, 1)
                tail_b(ps2b, pend)
                emit_sweep(ps2b, len(sweep_items))
            prev_at_ref[0] = at_blk

            # ---------------- final block: out-projection sweep -------------
            if blk == NBLK - 1:
                with tc.tile_pool(name="ps3", bufs=2, space="PSUM") as ps3:
                    for tt in range(TPB):
                        tb = blk * TPB + tt
                        oto = opool.tile([P, 2048], BF16, tag="oto")
                        for jb2 in range(2):
                            op_ps = ps3.tile([P, 2, 512], F32, tag="op")
                            for half in range(2):
                                jsl = slice((2 * jb2 + half) * 512,
                                            (2 * jb2 + half + 1) * 512)
                                for h in range(HPC):
                                    mm(op_ps[:, half], at_blk[:, h, tt, :],
                                       wo_sb[:, h, jsl], h == 0, h == HPC - 1)
                            nc.vector.tensor_copy(
                                oto[:, jb2 * 1024:(jb2 + 1) * 1024], op_ps[:])
                        nc.sync.dma_start(out=out_r[tb], in_=oto[:])

    nc.compile()
    return nc


_CACHE = {}


def _get_program():
    if "nc" not in _CACHE:
        _CACHE["nc"] = _build_program()
    return _CACHE["nc"]


def _host_inputs(hidden_states, Wq, Wk, Wv, Wo):
    """Fold rope+scale into weights, build per-core input maps."""
    f64 = np.float64
    mats = _rope_fold()
    scale = D ** -0.5
    Wq_f = np.empty((HID, HID), dtype=np.float32)
    for h in range(H):
        Wq_f[h * D:(h + 1) * D] = (mats[h] @ Wq[h * D:(h + 1) * D].astype(f64)
                                   * scale).astype(np.float32)
    perm = np.concatenate([np.arange(0, 64, 2), np.arange(1, 64, 2),
                           np.arange(64, 128, 2), np.arange(65, 128, 2)])
    Wk_f = Wk[perm].astype(np.float32)

    wkvT = np.ascontiguousarray(
        np.concatenate([Wk_f.T, Wv.T], axis=1)).astype(np.float16)
    ii = np.arange(P)[:, None]
    jj = np.arange(P)[None, :]
    dmask = (ii <= jj).astype(ml_dtypes.bfloat16)
    sel = np.zeros((P, P), dtype=np.float32)
    sel[[0, 32, 64, 96], 0] = 1.0

    hsT = [np.ascontiguousarray(hidden_states[b].T).astype(np.float16)
           for b in range(B)]
    in_maps = []
    for c in range(NCORES):
        b, q = c // CPB, c % CPB
        rows = slice(q * HD_PC, (q + 1) * HD_PC)
        in_maps.append({
            "hsT": hsT[b],
            "wqT": np.ascontiguousarray(Wq_f[rows].T).astype(np.float16),
            "wkvT": wkvT,
            "woT": np.ascontiguousarray(Wo[:, rows].T).astype(ml_dtypes.bfloat16),
            "dmask": dmask,
            "onbf": np.ones((P, P), dtype=ml_dtypes.bfloat16),
            "seld": sel.astype(ml_dtypes.bfloat16),
        })
    return in_maps


def kernel(hidden_states, Wq, Wk, Wv, Wo):
    hidden_states = np.asarray(hidden_states, dtype=np.float32)
    Wq = np.asarray(Wq, dtype=np.float32)
    Wk = np.asarray(Wk, dtype=np.float32)
    Wv = np.asarray(Wv, dtype=np.float32)
    Wo = np.asarray(Wo, dtype=np.float32)

    nc = _get_program()
    in_maps = _host_inputs(hidden_states, Wq, Wk, Wv, Wo)
    res = run_bass_kernel_spmd(nc, in_maps, list(range(NCORES)))
    parts = [np.asarray(r["out"]).astype(np.float32) for r in res.results]
    out = np.empty((B, T, HID), dtype=np.float32)
    for b in range(B):
        out[b] = parts[CPB * b]
        for q in range(1, CPB):
            out[b] += parts[CPB * b + q]
    return out


# revision 14
# speedup vs baseline: 1.0773x; 1.0773x over previous
"""MQA kernel for Trainium2 (8 NeuronCores, SPMD via bass/Tile).

Problem: nn_MultiQueryAttention (B=2, T=2048, HID=2048, H=16, D=128).

Key algebraic simplification: the reference's apply_rope treats q's layout
as (B,T,H,D) while q is actually (B,H,T,D), so the "position" axis is the
head index -> per-head rotation R_h acting on the D axis only, independent
of sequence position. R_h is folded into Wq on the host. k's rope at pos=0
is a pure channel permutation, folded into Wk. The score scale 1/sqrt(D)
is folded into Wq as well. What remains on-device is a plain causal MQA.

Sharding (uniform SPMD program, per-core data differs):
  core c -> batch c//4, heads (c%4)*4..(c%4)*4+3, full T.
  Each core: Q^T/K^T/V projections, causal softmax attention for its 4
  heads, and a partial out-projection (its heads' rows of Wo^T). The 4
  partials per batch are summed on the host.

Single-pass pipeline over 4 t-blocks of 512: project Q/K/V for the block,
run the 4 causal attention rows that became computable, then the block's
out-projection as one dense matmul sweep. hs-tile DMA for block b+1
overlaps block b (prefetch depth 16 ~ all DMA engines); DMA issue is
spread over the gpsimd (hs), scalar (Wq) and sync (Wk/Wv/Wo, outputs)
queues since each issue costs ~0.6us of queue time.

Precision: the PE streams 1 column/cycle for 2-byte dtypes with fast
weight loads; the Q/K path (hs, Wq, Wk/Wv, Q^T, K^T) runs in fp16 whose
11-bit mantissa keeps softmax scores accurate; probs/V/out-proj run in
bf16 (exp(40) range); all matmuls accumulate fp32 in PSUM.

Attention processes all 4 heads per matmul; scores for two key tiles land
in one 2-bank PSUM tile so a single 1024-wide ACT exp covers both (exp
would otherwise out-pace the PE). Softmax denominators come from
column-tiled M=1 ones-matmuls: 4 strips at tile_position (0,32j) run
concurrently in the PE array, so each exp tile costs ~1/4 matmul instead
of a full 512-column pass; a select-column matmul recombines the strips.
Each row's tail (reciprocal, rank-1 1/den broadcast matmul, normalize
into the block's at tile) is emitted inside the NEXT row so the PE never
waits on the DVE chain.
"""

import numpy as np
import ml_dtypes
from contextlib import ExitStack

import concourse.bass as bass
import concourse.tile as tile
from concourse import bacc, mybir
from concourse.bass_utils import run_bass_kernel_spmd
from concourse.masks import make_identity

F32 = mybir.dt.float32
FP16 = mybir.dt.float16
BF16 = mybir.dt.bfloat16
EXP = mybir.ActivationFunctionType.Exp

B, T, HID, H, D = 2, 2048, 2048, 16, 128
NCORES = 8
CPB = 4              # cores per batch
HPC = H // CPB       # 4 heads per core
HD_PC = HPC * D      # 512 output dims per core
P = 128
KT = T // P          # 16 key tiles
NK = HID // P        # 16 contraction tiles for projections
NBLK = 4             # t blocks of 512
TPB = 4              # query tiles per block


def _rope_fold():
    """Per-head rotation matrices R_h (128x128) from the reference's quirky rope."""
    half = D // 2
    theta = 1.0 / (10000.0 ** (np.arange(0, half, 2, dtype=np.float64) / half))
    mats = []
    for h in range(H):
        R = np.zeros((D, D), dtype=np.float64)
        c = np.cos(h * theta)
        s = np.sin(h * theta)
        for j in range(32):
            R[j, 2 * j] = c[j]
            R[j, 2 * j + 1] = -s[j]
            R[32 + j, 2 * j] = s[j]
            R[32 + j, 2 * j + 1] = c[j]
            R[64 + j, 64 + 2 * j] = c[j]
            R[64 + j, 64 + 2 * j + 1] = -s[j]
            R[96 + j, 64 + 2 * j] = s[j]
            R[96 + j, 64 + 2 * j + 1] = c[j]
        mats.append(R)
    return mats


def _build_program():
    nc = bacc.Bacc("TRN2", target_bir_lowering=False, debug=False,
                   enable_asserts=False, num_devices=NCORES)

    hsT = nc.dram_tensor("hsT", [HID, T], FP16, kind="ExternalInput").ap()
    wqT = nc.dram_tensor("wqT", [HID, HD_PC], FP16, kind="ExternalInput").ap()
    wkvT = nc.dram_tensor("wkvT", [HID, 2 * D], FP16, kind="ExternalInput").ap()
    woT = nc.dram_tensor("woT", [HD_PC, HID], BF16, kind="ExternalInput").ap()
    dmd = nc.dram_tensor("dmask", [P, P], BF16, kind="ExternalInput").ap()
    onbf = nc.dram_tensor("onbf", [P, P], BF16, kind="ExternalInput").ap()
    seld = nc.dram_tensor("seld", [P, P], BF16, kind="ExternalInput").ap()
    out = nc.dram_tensor("out", [T, HID], BF16, kind="ExternalOutput").ap()

    hsT_r = hsT.rearrange("(ko p) t -> ko p t", p=P)        # [16,128,2048]
    wqT_r = wqT.rearrange("(ko p) m -> p ko m", p=P)        # [128,16,512]
    wkvT_r = wkvT.rearrange("(ko p) d -> p ko d", p=P)      # [128,16,256]
    woT_r = woT.rearrange("(h p) n -> p h n", p=P)          # [128,4,2048]
    out_r = out.rearrange("(tt p) n -> tt p n", p=P)        # [16,128,2048]

    def mm(ps, lhsT, rhs, start, stop):
        nc.tensor.matmul(ps, lhsT=lhsT, rhs=rhs, start=start, stop=stop)

    with tile.TileContext(nc) as tc, ExitStack() as ctx:
        singles = ctx.enter_context(tc.tile_pool(name="singles", bufs=1))
        hpool = ctx.enter_context(tc.tile_pool(name="hst", bufs=16))
        epool = ctx.enter_context(tc.tile_pool(name="etile", bufs=6))
        spool = ctx.enter_context(tc.tile_pool(name="small", bufs=2))
        apool = ctx.enter_context(tc.tile_pool(name="att", bufs=2))
        opool = ctx.enter_context(tc.tile_pool(name="outt", bufs=4))

        ident = singles.tile([P, P], F32)
        make_identity(nc, ident)
        ident_bf = singles.tile([P, P], BF16)
        nc.vector.tensor_copy(ident_bf[:], ident[:])
        dmask = singles.tile([P, P], BF16)
        ones_bf = singles.tile([P, P], BF16)
        sel_bf = singles.tile([P, P], BF16)
        nc.sync.dma_start(out=dmask, in_=dmd)
        nc.sync.dma_start(out=ones_bf, in_=onbf)
        nc.sync.dma_start(out=sel_bf, in_=seld)

        # weight residents; per-k slices stream in with the first block's
        # data. Wo rides the sync queue (outputs only start later).
        wq_sb = singles.tile([P, NK, HD_PC], FP16)
        wkv_sb = singles.tile([P, NK, 2 * D], FP16)
        wo_sb = singles.tile([P, HPC, HID], BF16)

        # resident activations
        qt_sb = singles.tile([P, HPC, T], FP16)      # Q^T per head [d, t]
        kt_sb = singles.tile([P, T], FP16)           # K^T [d, s]
        v_sb = singles.tile([P, KT, D], BF16)        # V natural [s-tile, d]

        dmask_b = dmask[:, None, :].to_broadcast([P, HPC, P])
        prev_at_ref = [None]  # at tile of the previous block (sweep deferral)

        def tail_a(pp, st_):
            """First half of a row's softmax tail: recombine strips, 1/den."""
            den_ps, strips = st_["den_ps"], st_["strips"]
            if strips:  # recombine the 4 column-tiled strip rows
                dstr = spool.tile([P, HPC, P], BF16, tag="dstr", name="dstr")
                nc.vector.tensor_copy(dstr[:], den_ps[:])
                dt_ps = pp.tile([P, HPC, P], F32, tag="pp", name="dt_ps")
                mm(dt_ps[:1], sel_bf[:, :1], dstr[:], True, True)
                den_ap = dt_ps[:1]
            else:
                den_ap = den_ps[:1]
            recip = spool.tile([1, HPC, P], F32, tag="recip", name="recip")
            nc.vector.reciprocal_approx_fast(out=recip[:], in_=den_ap)
            recr = spool.tile([1, HPC, P], BF16, tag="recr", name="recr")
            nc.vector.tensor_copy(recr[:], recip[:])
            st_["recr"] = recr

        def tail_b(pp, st_):
            """Second half: broadcast 1/den across partitions, normalize O^T."""
            bc_ps = pp.tile([P, HPC, P], F32, tag="pp", name="bc_ps")
            mm(bc_ps[:], ones_bf[:1, :], st_["recr"][:], True, True)
            bc_sb = spool.tile([P, HPC, P], F32, tag="bc_sb", name="bc_sb")
            nc.vector.tensor_copy(bc_sb[:], bc_ps[:])
            nc.vector.tensor_mul(st_["at_blk"][:, :, st_["tt"], :],
                                 st_["ot_ps"][:], bc_sb[:])

        for blk in range(NBLK):
            tsl = slice(blk * 512, (blk + 1) * 512)
            # ---------------- projection for this 512-block ----------------
            with tc.tile_pool(name="ps1", bufs=1, space="PSUM") as ps1, \
                 tc.tile_pool(name="ps1t", bufs=2, space="PSUM") as ps1t:
                q_ps = [ps1.tile([P, 512], F32, tag=f"qps{h}", name=f"qps{h}")
                        for h in range(HPC)]
                k_ps = ps1.tile([P, 512], F32, tag="kps")
                v_ps = ps1.tile([P, 512], F32, tag="vps")
                for k in range(NK):
                    if blk == 0:  # stream weight slices just ahead of data
                        nc.sync.dma_start(out=wkv_sb[:, k, :], in_=wkvT_r[:, k, :])
                        nc.scalar.dma_start(out=wq_sb[:, k, :], in_=wqT_r[:, k, :])
                    hst = hpool.tile([P, 512], FP16)
                    nc.gpsimd.dma_start(out=hst, in_=hsT_r[k][:, tsl])
                    st, sp = (k == 0), (k == NK - 1)
                    for h in range(HPC):
                        mm(q_ps[h][:], wq_sb[:, k, h * D:(h + 1) * D], hst[:], st, sp)
                    mm(k_ps[:], wkv_sb[:, k, :D], hst[:], st, sp)
                    mm(v_ps[:], wkv_sb[:, k, D:], hst[:], st, sp)
                if blk == 0:  # out-proj weights: first needed at block 0's sweep
                    for h in range(HPC):
                        for jb in range(4):
                            nc.sync.dma_start(
                                out=wo_sb[:, h, jb * 512:(jb + 1) * 512],
                                in_=woT_r[:, h, jb * 512:(jb + 1) * 512])
                # V^T -> V natural via PE transpose (bf16); vt copy first so
                # the transposes start while the qt copies drain, and qt in
                # row-tile slices so row blk*4's scores start after 4 copies.
                vt_sb = spool.tile([P, 512], BF16, tag="vt")
                nc.vector.tensor_copy(vt_sb[:], v_ps[:])
                for tt in range(TPB):
                    for h in range(HPC):
                        qsl = slice(blk * 512 + tt * P, blk * 512 + (tt + 1) * P)
                        nc.vector.tensor_copy(qt_sb[:, h, qsl],
                                              q_ps[h][:, tt * P:(tt + 1) * P])
                nc.vector.tensor_copy(kt_sb[:, tsl], k_ps[:])
                for si in range(4):
                    pt = ps1t.tile([P, P], BF16, tag="tps")
                    nc.tensor.transpose(pt[:], vt_sb[:, si * P:(si + 1) * P],
                                        ident_bf[:])
                    nc.vector.tensor_copy(v_sb[:, blk * 4 + si, :], pt[:])

            # ---------------- attention rows of this block -----------------
            # Software-pipelined pair stream: scores+exp for pair i+1 are
            # emitted before AV/den of pair i, so the PE never sits behind
            # an AV that waits on the ACT exp chain. Row tails (two PE ops +
            # DVE chain) advance one stage per pair, ahead of the AVs that
            # would deadlock on them.
            at_blk = apool.tile([P, HPC, TPB, P], BF16, tag="atb", name="at_blk")
            pairs = []  # (tt, st0, npair)
            for tt in range(TPB):
                tb = blk * TPB + tt
                for st0 in range(0, tb + 1, 2):
                    pairs.append((tt, st0, min(2, tb + 1 - st0)))
            rowstate = {}
            tails = []  # pend dicts: tail_a then tail_b, one stage per pair

            with tc.tile_pool(name="ps2s", bufs=2, space="PSUM") as ps2s, \
                 tc.tile_pool(name="ps2o", bufs=2, space="PSUM") as ps2o, \
                 tc.tile_pool(name="ps2d", bufs=1, space="PSUM") as ps2d, \
                 tc.tile_pool(name="ps2b", bufs=1, space="PSUM") as ps2b:

                def emit_scores(pr):
                    tt, st0, npair = pr
                    tb = blk * TPB + tt
                    qrhs = qt_sb[:, :, tb * P:(tb + 1) * P]
                    s_ps = ps2s.tile([P, 2, HPC, P], F32, tag="sps", name="s_ps")
                    for i in range(npair):
                        st = st0 + i
                        mm(s_ps[:, i], kt_sb[:, st * P:(st + 1) * P],
                           qrhs, True, True)
                    e_sb = epool.tile([P, 2, HPC, P], BF16, tag="etile",
                                      name="e_sb")
                    if npair == 2:
                        nc.scalar.activation(e_sb[:], s_ps[:], EXP)
                    else:
                        nc.scalar.activation(e_sb[:, 0], s_ps[:, 0], EXP)
                    if st0 + npair - 1 == tb:  # diagonal tile: causal mask
                        nc.vector.tensor_mul(e_sb[:, npair - 1],
                                             e_sb[:, npair - 1], dmask_b)
                    return e_sb

                def emit_tail_stage():
                    if not tails:
                        return
                    st_ = tails[0]
                    if st_["stage"] == 0:
                        tail_a(ps2b, st_)
                        st_["stage"] = 1
                    else:
                        tail_b(ps2b, st_)
                        tails.pop(0)

                def emit_avden(pr, e_sb):
                    tt, st0, npair = pr
                    tb = blk * TPB + tt
                    strips = tb >= 3
                    if tt not in rowstate:
                        rowstate[tt] = {
                            "ot_ps": ps2o.tile([P, HPC, P], F32, tag="ot",
                                               name="ot_ps"),
                            "den_ps": ps2d.tile([P, HPC, P], F32, tag="den",
                                                name="den_ps"),
                            "strips": strips, "pstrips": [],
                            "at_blk": at_blk, "tt": tt, "stage": 0,
                        }
                    rs = rowstate[tt]
                    for i in range(npair):
                        st = st0 + i
                        if strips:
                            rs["pstrips"].append((st, e_sb[:, i]))
                        else:
                            mm(rs["den_ps"][:1], ones_bf[:, :1], e_sb[:, i],
                               st == 0, st == tb)
                        mm(rs["ot_ps"][:], v_sb[:, st, :], e_sb[:, i],
                           st == 0, st == tb)
                    # den strips: back-to-back groups of 4 run concurrently
                    # in distinct PE column groups
                    if len(rs["pstrips"]) >= 4 or st0 + npair - 1 == tb:
                        for st, e_ap in rs["pstrips"]:
                            j = st % 4
                            nc.tensor.matmul(
                                rs["den_ps"][32 * j:32 * j + 1, :],
                                lhsT=ones_bf[:, :1], rhs=e_ap,
                                start=(st < 4), stop=(st + 4 > tb),
                                tile_position=(0, 32 * j))
                        rs["pstrips"] = []
                    if st0 + npair - 1 == tb:  # row complete -> queue tail
                        tails.append(rs)

                e_cur = emit_scores(pairs[0])
                for i, pr in enumerate(pairs):
                    e_next = emit_scores(pairs[i + 1]) if i + 1 < len(pairs) \
                        else None
                    emit_tail_stage()
                    emit_avden(pr, e_cur)
                    e_cur = e_next
                while tails:
                    emit_tail_stage()

            # ---------------- out-projection sweep for this block ----------
            with tc.tile_pool(name="ps3", bufs=2, space="PSUM") as ps3:
                for tt in range(TPB):
                    tb = blk * TPB + tt
                    oto = opool.tile([P, 2048], BF16, tag="oto")
                    for jb2 in range(2):
                        op_ps = ps3.tile([P, 2, 512], F32, tag="op")
                        for half in range(2):
                            jsl = slice((2 * jb2 + half) * 512,
                                        (2 * jb2 + half + 1) * 512)
                            for h in range(HPC):
                                mm(op_ps[:, half], at_blk[:, h, tt, :],
                                   wo_sb[:, h, jsl], h == 0, h == HPC - 1)
                        nc.vector.tensor_copy(
                            oto[:, jb2 * 1024:(jb2 + 1) * 1024], op_ps[:])
                    nc.sync.dma_start(out=out_r[tb], in_=oto[:])

    nc.compile()
    return nc


_CACHE = {}


def _get_program():
    if "nc" not in _CACHE:
        _CACHE["nc"] = _build_program()
    return _CACHE["nc"]


def _host_inputs(hidden_states, Wq, Wk, Wv, Wo):
    """Fold rope+scale into weights, build per-core input maps."""
    f64 = np.float64
    mats = _rope_fold()
    scale = D ** -0.5
    Wq_f = np.empty((HID, HID), dtype=np.float32)
    for h in range(H):
        Wq_f[h * D:(h + 1) * D] = (mats[h] @ Wq[h * D:(h + 1) * D].astype(f64)
                                   * scale).astype(np.float32)
    perm = np.concatenate([np.arange(0, 64, 2), np.arange(1, 64, 2),
                           np.arange(64, 128, 2), np.arange(65, 128, 2)])
    Wk_f = Wk[perm].astype(np.float32)

    wkvT = np.ascontiguousarray(
        np.concatenate([Wk_f.T, Wv.T], axis=1)).astype(np.float16)
    ii = np.arange(P)[:, None]
    jj = np.arange(P)[None, :]
    dmask = (ii <= jj).astype(ml_dtypes.bfloat16)
    sel = np.zeros((P, P), dtype=np.float32)
    sel[[0, 32, 64, 96], 0] = 1.0

    hsT = [np.ascontiguousarray(hidden_states[b].T).astype(np.float16)
           for b in range(B)]
    in_maps = []
    for c in range(NCORES):
        b, q = c // CPB, c % CPB
        rows = slice(q * HD_PC, (q + 1) * HD_PC)
        in_maps.append({
            "hsT": hsT[b],
            "wqT": np.ascontiguousarray(Wq_f[rows].T).astype(np.float16),
            "wkvT": wkvT,
            "woT": np.ascontiguousarray(Wo[:, rows].T).astype(ml_dtypes.bfloat16),
            "dmask": dmask,
            "onbf": np.ones((P, P), dtype=ml_dtypes.bfloat16),
            "seld": sel.astype(ml_dtypes.bfloat16),
        })
    return in_maps


def kernel(hidden_states, Wq, Wk, Wv, Wo):
    hidden_states = np.asarray(hidden_states, dtype=np.float32)
    Wq = np.asarray(Wq, dtype=np.float32)
    Wk = np.asarray(Wk, dtype=np.float32)
    Wv = np.asarray(Wv, dtype=np.float32)
    Wo = np.asarray(Wo, dtype=np.float32)

    nc = _get_program()
    in_maps = _host_inputs(hidden_states, Wq, Wk, Wv, Wo)
    res = run_bass_kernel_spmd(nc, in_maps, list(range(NCORES)))
    parts = [np.asarray(r["out"]).astype(np.float32) for r in res.results]
    out = np.empty((B, T, HID), dtype=np.float32)
    for b in range(B):
        out[b] = parts[CPB * b]
        for q in range(1, CPB):
            out[b] += parts[CPB * b + q]
    return out


# revision 15
# speedup vs baseline: 1.1207x; 1.0403x over previous
"""MQA kernel for Trainium2 (8 NeuronCores, SPMD via bass/Tile).

Problem: nn_MultiQueryAttention (B=2, T=2048, HID=2048, H=16, D=128).

Key algebraic simplification: the reference's apply_rope treats q's layout
as (B,T,H,D) while q is actually (B,H,T,D), so the "position" axis is the
head index -> per-head rotation R_h acting on the D axis only, independent
of sequence position. R_h is folded into Wq on the host. k's rope at pos=0
is a pure channel permutation, folded into Wk. The score scale 1/sqrt(D)
is folded into Wq as well. What remains on-device is a plain causal MQA.

Sharding (uniform SPMD program, per-core data differs):
  core c -> batch c//4, heads (c%4)*4..(c%4)*4+3, full T.
  Each core: Q^T/K^T/V projections, causal softmax attention for its 4
  heads, and a partial out-projection (its heads' rows of Wo^T). The 4
  partials per batch are summed on the host.

Single-pass pipeline over 4 t-blocks of 512: project Q/K/V for the block,
run the 4 causal attention rows that became computable, then the block's
out-projection as one dense matmul sweep. hs-tile DMA for block b+1
overlaps block b (prefetch depth 16 ~ all DMA engines); DMA issue is
spread over the gpsimd (hs), scalar (Wq) and sync (Wk/Wv/Wo, outputs)
queues since each issue costs ~0.6us of queue time.

Precision: the PE streams 1 column/cycle for 2-byte dtypes with fast
weight loads; the Q/K path (hs, Wq, Wk/Wv, Q^T, K^T) runs in fp16 whose
11-bit mantissa keeps softmax scores accurate; probs/V/out-proj run in
bf16 (exp(40) range); all matmuls accumulate fp32 in PSUM.

Attention processes all 4 heads per matmul; scores for two key tiles land
in one 2-bank PSUM tile so a single 1024-wide ACT exp covers both (exp
would otherwise out-pace the PE). Softmax denominators come from
column-tiled M=1 ones-matmuls: 4 strips at tile_position (0,32j) run
concurrently in the PE array, so each exp tile costs ~1/4 matmul instead
of a full 512-column pass; a select-column matmul recombines the strips.
Each row's tail (reciprocal, rank-1 1/den broadcast matmul, normalize
into the block's at tile) is emitted inside the NEXT row so the PE never
waits on the DVE chain.
"""

import numpy as np
import ml_dtypes
from contextlib import ExitStack

import concourse.bass as bass
import concourse.tile as tile
from concourse import bacc, mybir
from concourse.bass_utils import run_bass_kernel_spmd
from concourse.masks import make_identity

F32 = mybir.dt.float32
FP16 = mybir.dt.float16
BF16 = mybir.dt.bfloat16
EXP = mybir.ActivationFunctionType.Exp

B, T, HID, H, D = 2, 2048, 2048, 16, 128
NCORES = 8
CPB = 4              # cores per batch
HPC = H // CPB       # 4 heads per core
HD_PC = HPC * D      # 512 output dims per core
P = 128
KT = T // P          # 16 key tiles
NK = HID // P        # 16 contraction tiles for projections
NBLK = 4             # t blocks of 512
TPB = 4              # query tiles per block


def _rope_fold():
    """Per-head rotation matrices R_h (128x128) from the reference's quirky rope."""
    half = D // 2
    theta = 1.0 / (10000.0 ** (np.arange(0, half, 2, dtype=np.float64) / half))
    mats = []
    for h in range(H):
        R = np.zeros((D, D), dtype=np.float64)
        c = np.cos(h * theta)
        s = np.sin(h * theta)
        for j in range(32):
            R[j, 2 * j] = c[j]
            R[j, 2 * j + 1] = -s[j]
            R[32 + j, 2 * j] = s[j]
            R[32 + j, 2 * j + 1] = c[j]
            R[64 + j, 64 + 2 * j] = c[j]
            R[64 + j, 64 + 2 * j + 1] = -s[j]
            R[96 + j, 64 + 2 * j] = s[j]
            R[96 + j, 64 + 2 * j + 1] = c[j]
        mats.append(R)
    return mats


def _build_program():
    nc = bacc.Bacc("TRN2", target_bir_lowering=False, debug=False,
                   enable_asserts=False, num_devices=NCORES)

    hsT = nc.dram_tensor("hsT", [HID, T], FP16, kind="ExternalInput").ap()
    wqT = nc.dram_tensor("wqT", [HID, HD_PC], FP16, kind="ExternalInput").ap()
    wkvT = nc.dram_tensor("wkvT", [HID, 2 * D], FP16, kind="ExternalInput").ap()
    woT = nc.dram_tensor("woT", [HD_PC, HID], BF16, kind="ExternalInput").ap()
    dmd = nc.dram_tensor("dmask", [P, P], BF16, kind="ExternalInput").ap()
    onbf = nc.dram_tensor("onbf", [P, P], BF16, kind="ExternalInput").ap()
    seld = nc.dram_tensor("seld", [P, P], BF16, kind="ExternalInput").ap()
    out = nc.dram_tensor("out", [T, HID], BF16, kind="ExternalOutput").ap()

    hsT_r = hsT.rearrange("(ko p) t -> ko p t", p=P)        # [16,128,2048]
    wqT_r = wqT.rearrange("(ko p) m -> p ko m", p=P)        # [128,16,512]
    wkvT_r = wkvT.rearrange("(ko p) d -> p ko d", p=P)      # [128,16,256]
    woT_r = woT.rearrange("(h p) n -> p h n", p=P)          # [128,4,2048]
    out_r = out.rearrange("(tt p) n -> tt p n", p=P)        # [16,128,2048]

    def mm(ps, lhsT, rhs, start, stop):
        nc.tensor.matmul(ps, lhsT=lhsT, rhs=rhs, start=start, stop=stop)

    with tile.TileContext(nc) as tc, ExitStack() as ctx:
        singles = ctx.enter_context(tc.tile_pool(name="singles", bufs=1))
        hpool = ctx.enter_context(tc.tile_pool(name="hst", bufs=16))
        epool = ctx.enter_context(tc.tile_pool(name="etile", bufs=6))
        spool = ctx.enter_context(tc.tile_pool(name="small", bufs=2))
        apool = ctx.enter_context(tc.tile_pool(name="att", bufs=2))
        opool = ctx.enter_context(tc.tile_pool(name="outt", bufs=4))

        ident = singles.tile([P, P], F32)
        make_identity(nc, ident)
        ident_bf = singles.tile([P, P], BF16)
        nc.vector.tensor_copy(ident_bf[:], ident[:])
        dmask = singles.tile([P, P], BF16)
        ones_bf = singles.tile([P, P], BF16)
        sel_bf = singles.tile([P, P], BF16)
        nc.sync.dma_start(out=dmask, in_=dmd)
        nc.sync.dma_start(out=ones_bf, in_=onbf)
        nc.sync.dma_start(out=sel_bf, in_=seld)

        # weight residents; per-k slices stream in with the first block's
        # data. Wo rides the sync queue (outputs only start later).
        wq_sb = singles.tile([P, NK, HD_PC], FP16)
        wkv_sb = singles.tile([P, NK, 2 * D], FP16)
        wo_sb = singles.tile([P, HPC, HID], BF16)

        # resident activations
        qt_sb = singles.tile([P, HPC, T], FP16)      # Q^T per head [d, t]
        kt_sb = singles.tile([P, T], FP16)           # K^T [d, s]
        v_sb = singles.tile([P, KT, D], BF16)        # V natural [s-tile, d]

        dmask_b = dmask[:, None, :].to_broadcast([P, HPC, P])
        prev_at_ref = [None]  # at tile of the previous block (sweep deferral)

        def tail_a(pp, st_):
            """First half of a row's softmax tail: recombine strips, 1/den."""
            den_ps, strips = st_["den_ps"], st_["strips"]
            if strips:  # recombine the 4 column-tiled strip rows
                dstr = spool.tile([P, HPC, P], BF16, tag="dstr", name="dstr")
                nc.vector.tensor_copy(dstr[:], den_ps[:])
                dt_ps = pp.tile([P, HPC, P], F32, tag="pp", name="dt_ps")
                mm(dt_ps[:1], sel_bf[:, :1], dstr[:], True, True)
                den_ap = dt_ps[:1]
            else:
                den_ap = den_ps[:1]
            recip = spool.tile([1, HPC, P], F32, tag="recip", name="recip")
            nc.vector.reciprocal_approx_fast(out=recip[:], in_=den_ap)
            recr = spool.tile([1, HPC, P], BF16, tag="recr", name="recr")
            nc.vector.tensor_copy(recr[:], recip[:])
            st_["recr"] = recr

        def tail_b(pp, st_):
            """Second half: broadcast 1/den across partitions, normalize O^T."""
            bc_ps = pp.tile([P, HPC, P], F32, tag="pp", name="bc_ps")
            mm(bc_ps[:], ones_bf[:1, :], st_["recr"][:], True, True)
            bc_sb = spool.tile([P, HPC, P], F32, tag="bc_sb", name="bc_sb")
            nc.vector.tensor_copy(bc_sb[:], bc_ps[:])
            nc.vector.tensor_mul(st_["at_blk"][:, :, st_["tt"], :],
                                 st_["ot_ps"][:], bc_sb[:])

        for blk in range(NBLK):
            tsl = slice(blk * 512, (blk + 1) * 512)
            # ---------------- projection for this 512-block ----------------
            with tc.tile_pool(name="ps1", bufs=1, space="PSUM") as ps1:
                q_ps = [ps1.tile([P, 512], F32, tag=f"qps{h}", name=f"qps{h}")
                        for h in range(HPC)]
                k_ps = ps1.tile([P, 512], F32, tag="kps")
                v_ps = ps1.tile([P, 512], F32, tag="vps")
                for k in range(NK):
                    if blk == 0:  # stream weight slices just ahead of data
                        nc.sync.dma_start(out=wkv_sb[:, k, :], in_=wkvT_r[:, k, :])
                        nc.scalar.dma_start(out=wq_sb[:, k, :], in_=wqT_r[:, k, :])
                    hst = hpool.tile([P, 512], FP16)
                    nc.gpsimd.dma_start(out=hst, in_=hsT_r[k][:, tsl])
                    st, sp = (k == 0), (k == NK - 1)
                    for h in range(HPC):
                        mm(q_ps[h][:], wq_sb[:, k, h * D:(h + 1) * D], hst[:], st, sp)
                    mm(k_ps[:], wkv_sb[:, k, :D], hst[:], st, sp)
                    mm(v_ps[:], wkv_sb[:, k, D:], hst[:], st, sp)
                if blk == 0:  # out-proj weights: first needed at block 0's sweep
                    for h in range(HPC):
                        for jb in range(4):
                            nc.sync.dma_start(
                                out=wo_sb[:, h, jb * 512:(jb + 1) * 512],
                                in_=woT_r[:, h, jb * 512:(jb + 1) * 512])
                # vt copy first so the V transposes (emitted in the rows
                # section) can start early; qt in row-tile slices so row
                # blk*4's scores start after 4 small copies.
                vt_sb = spool.tile([P, 512], BF16, tag="vt")
                nc.vector.tensor_copy(vt_sb[:], v_ps[:])
                for tt in range(TPB):
                    for h in range(HPC):
                        qsl = slice(blk * 512 + tt * P, blk * 512 + (tt + 1) * P)
                        nc.vector.tensor_copy(qt_sb[:, h, qsl],
                                              q_ps[h][:, tt * P:(tt + 1) * P])
                nc.vector.tensor_copy(kt_sb[:, tsl], k_ps[:])

            # ---------------- attention rows of this block -----------------
            # Software-pipelined pair stream: scores+exp for pair i+1 are
            # emitted before AV/den of pair i, so the PE never sits behind
            # an AV that waits on the ACT exp chain. Row tails (two PE ops +
            # DVE chain) advance one stage per pair, ahead of the AVs that
            # would deadlock on them.
            at_blk = apool.tile([P, HPC, TPB, P], BF16, tag="atb", name="at_blk")
            pairs = []  # (tt, st0, npair)
            for tt in range(TPB):
                tb = blk * TPB + tt
                for st0 in range(0, tb + 1, 2):
                    pairs.append((tt, st0, min(2, tb + 1 - st0)))
            rowstate = {}
            tails = []  # pend dicts: tail_a then tail_b, one stage per pair

            with tc.tile_pool(name="ps2s", bufs=2, space="PSUM") as ps2s, \
                 tc.tile_pool(name="ps2o", bufs=2, space="PSUM") as ps2o, \
                 tc.tile_pool(name="ps2d", bufs=1, space="PSUM") as ps2d, \
                 tc.tile_pool(name="ps2b", bufs=1, space="PSUM") as ps2b:

                def emit_scores(pr):
                    tt, st0, npair = pr
                    tb = blk * TPB + tt
                    qrhs = qt_sb[:, :, tb * P:(tb + 1) * P]
                    s_ps = ps2s.tile([P, 2, HPC, P], F32, tag="sps", name="s_ps")
                    for i in range(npair):
                        st = st0 + i
                        mm(s_ps[:, i], kt_sb[:, st * P:(st + 1) * P],
                           qrhs, True, True)
                    e_sb = epool.tile([P, 2, HPC, P], BF16, tag="etile",
                                      name="e_sb")
                    if npair == 2:
                        nc.scalar.activation(e_sb[:], s_ps[:], EXP)
                    else:
                        nc.scalar.activation(e_sb[:, 0], s_ps[:, 0], EXP)
                    if st0 + npair - 1 == tb:  # diagonal tile: causal mask
                        nc.vector.tensor_mul(e_sb[:, npair - 1],
                                             e_sb[:, npair - 1], dmask_b)
                    return e_sb

                def emit_tail_stage():
                    if not tails:
                        return
                    st_ = tails[0]
                    if st_["stage"] == 0:
                        tail_a(ps2b, st_)
                        st_["stage"] = 1
                    else:
                        tail_b(ps2b, st_)
                        tails.pop(0)

                def emit_avden(pr, e_sb):
                    tt, st0, npair = pr
                    tb = blk * TPB + tt
                    strips = tb >= 3
                    if tt not in rowstate:
                        rowstate[tt] = {
                            "ot_ps": ps2o.tile([P, HPC, P], F32, tag="ot",
                                               name="ot_ps"),
                            "den_ps": ps2d.tile([P, HPC, P], F32, tag="den",
                                                name="den_ps"),
                            "strips": strips, "pstrips": [],
                            "at_blk": at_blk, "tt": tt, "stage": 0,
                        }
                    rs = rowstate[tt]
                    for i in range(npair):
                        st = st0 + i
                        if strips:
                            rs["pstrips"].append((st, e_sb[:, i]))
                        else:
                            mm(rs["den_ps"][:1], ones_bf[:, :1], e_sb[:, i],
                               st == 0, st == tb)
                        mm(rs["ot_ps"][:], v_sb[:, st, :], e_sb[:, i],
                           st == 0, st == tb)
                    # den strips: back-to-back groups of 4 run concurrently
                    # in distinct PE column groups
                    if len(rs["pstrips"]) >= 4 or st0 + npair - 1 == tb:
                        for st, e_ap in rs["pstrips"]:
                            j = st % 4
                            nc.tensor.matmul(
                                rs["den_ps"][32 * j:32 * j + 1, :],
                                lhsT=ones_bf[:, :1], rhs=e_ap,
                                start=(st < 4), stop=(st + 4 > tb),
                                tile_position=(0, 32 * j))
                        rs["pstrips"] = []
                    if st0 + npair - 1 == tb:  # row complete -> queue tail
                        tails.append(rs)

                # V^T -> V natural via PE transpose (bf16), interleaved
                # with the first scores so the block transition has no
                # PE bubble; AVs need this block's v only at its diagonal.
                e_q = [emit_scores(pairs[0])]
                for si in range(4):
                    pt = ps2b.tile([P, HPC, P], BF16, tag="pp", name="pt")
                    nc.tensor.transpose(pt[:, 0, :],
                                        vt_sb[:, si * P:(si + 1) * P],
                                        ident_bf[:])
                    nc.vector.tensor_copy(v_sb[:, blk * 4 + si, :],
                                          pt[:, 0, :])
                if len(pairs) > 1:
                    e_q.append(emit_scores(pairs[1]))
                for i, pr in enumerate(pairs):
                    if i + 2 < len(pairs):
                        e_q.append(emit_scores(pairs[i + 2]))
                    emit_tail_stage()
                    emit_avden(pr, e_q.pop(0))
                while tails:
                    emit_tail_stage()

            # ---------------- out-projection sweep for this block ----------
            with tc.tile_pool(name="ps3", bufs=2, space="PSUM") as ps3:
                for tt in range(TPB):
                    tb = blk * TPB + tt
                    oto = opool.tile([P, 2048], BF16, tag="oto")
                    for jb2 in range(2):
                        op_ps = ps3.tile([P, 2, 512], F32, tag="op")
                        for half in range(2):
                            jsl = slice((2 * jb2 + half) * 512,
                                        (2 * jb2 + half + 1) * 512)
                            for h in range(HPC):
                                mm(op_ps[:, half], at_blk[:, h, tt, :],
                                   wo_sb[:, h, jsl], h == 0, h == HPC - 1)
                        nc.vector.tensor_copy(
                            oto[:, jb2 * 1024:(jb2 + 1) * 1024], op_ps[:])
                    nc.sync.dma_start(out=out_r[tb], in_=oto[:])

    nc.compile()
    return nc


_CACHE = {}


def _get_program():
    if "nc" not in _CACHE:
        _CACHE["nc"] = _build_program()
    return _CACHE["nc"]


def _host_inputs(hidden_states, Wq, Wk, Wv, Wo):
    """Fold rope+scale into weights, build per-core input maps."""
    f64 = np.float64
    mats = _rope_fold()
    scale = D ** -0.5
    Wq_f = np.empty((HID, HID), dtype=np.float32)
    for h in range(H):
        Wq_f[h * D:(h + 1) * D] = (mats[h] @ Wq[h * D:(h + 1) * D].astype(f64)
                                   * scale).astype(np.float32)
    perm = np.concatenate([np.arange(0, 64, 2), np.arange(1, 64, 2),
                           np.arange(64, 128, 2), np.arange(65, 128, 2)])
    Wk_f = Wk[perm].astype(np.float32)

    wkvT = np.ascontiguousarray(
        np.concatenate([Wk_f.T, Wv.T], axis=1)).astype(np.float16)
    ii = np.arange(P)[:, None]
    jj = np.arange(P)[None, :]
    dmask = (ii <= jj).astype(ml_dtypes.bfloat16)
    sel = np.zeros((P, P), dtype=np.float32)
    sel[[0, 32, 64, 96], 0] = 1.0

    hsT = [np.ascontiguousarray(hidden_states[b].T).astype(np.float16)
           for b in range(B)]
    in_maps = []
    for c in range(NCORES):
        b, q = c // CPB, c % CPB
        rows = slice(q * HD_PC, (q + 1) * HD_PC)
        in_maps.append({
            "hsT": hsT[b],
            "wqT": np.ascontiguousarray(Wq_f[rows].T).astype(np.float16),
            "wkvT": wkvT,
            "woT": np.ascontiguousarray(Wo[:, rows].T).astype(ml_dtypes.bfloat16),
            "dmask": dmask,
            "onbf": np.ones((P, P), dtype=ml_dtypes.bfloat16),
            "seld": sel.astype(ml_dtypes.bfloat16),
        })
    return in_maps


def kernel(hidden_states, Wq, Wk, Wv, Wo):
    hidden_states = np.asarray(hidden_states, dtype=np.float32)
    Wq = np.asarray(Wq, dtype=np.float32)
    Wk = np.asarray(Wk, dtype=np.float32)
    Wv = np.asarray(Wv, dtype=np.float32)
    Wo = np.asarray(Wo, dtype=np.float32)

    nc = _get_program()
    in_maps = _host_inputs(hidden_states, Wq, Wk, Wv, Wo)
    res = run_bass_kernel_spmd(nc, in_maps, list(range(NCORES)))
    parts = [np.asarray(r["out"]).astype(np.float32) for r in res.results]
    out = np.empty((B, T, HID), dtype=np.float32)
    for b in range(B):
        out[b] = parts[CPB * b]
        for q in range(1, CPB):
            out[b] += parts[CPB * b + q]
    return out


# revision 16
# speedup vs baseline: 1.1211x; 1.0004x over previous
"""MQA kernel for Trainium2 (8 NeuronCores, SPMD via bass/Tile).

Problem: nn_MultiQueryAttention (B=2, T=2048, HID=2048, H=16, D=128).

Key algebraic simplification: the reference's apply_rope treats q's layout
as (B,T,H,D) while q is actually (B,H,T,D), so the "position" axis is the
head index -> per-head rotation R_h acting on the D axis only, independent
of sequence position. R_h is folded into Wq on the host. k's rope at pos=0
is a pure channel permutation, folded into Wk. The score scale 1/sqrt(D)
is folded into Wq as well. What remains on-device is a plain causal MQA.

Sharding (uniform SPMD program, per-core data differs):
  core c -> batch c//4, heads (c%4)*4..(c%4)*4+3, full T.
  Each core: Q^T/K^T/V projections, causal softmax attention for its 4
  heads, and a partial out-projection (its heads' rows of Wo^T). The 4
  partials per batch are summed on the host.

Single-pass pipeline over 4 t-blocks of 512: project Q/K/V for the block,
run the 4 causal attention rows that became computable, then the block's
out-projection as one dense matmul sweep. hs-tile DMA for block b+1
overlaps block b (prefetch depth 16 ~ all DMA engines); DMA issue is
spread over the gpsimd (hs), scalar (Wq) and sync (Wk/Wv/Wo, outputs)
queues since each issue costs ~0.6us of queue time.

Precision: the PE streams 1 column/cycle for 2-byte dtypes with fast
weight loads; the Q/K path (hs, Wq, Wk/Wv, Q^T, K^T) runs in fp16 whose
11-bit mantissa keeps softmax scores accurate; probs/V/out-proj run in
bf16 (exp(40) range); all matmuls accumulate fp32 in PSUM.

Attention processes all 4 heads per matmul; scores for two key tiles land
in one 2-bank PSUM tile so a single 1024-wide ACT exp covers both (exp
would otherwise out-pace the PE). Softmax denominators come from
column-tiled M=1 ones-matmuls: 4 strips at tile_position (0,32j) run
concurrently in the PE array, so each exp tile costs ~1/4 matmul instead
of a full 512-column pass; a select-column matmul recombines the strips.
Each row's tail (reciprocal, rank-1 1/den broadcast matmul, normalize
into the block's at tile) is emitted inside the NEXT row so the PE never
waits on the DVE chain.
"""

import numpy as np
import ml_dtypes
from contextlib import ExitStack

import concourse.bass as bass
import concourse.tile as tile
from concourse import bacc, mybir
from concourse.bass_utils import run_bass_kernel_spmd
from concourse.masks import make_identity

F32 = mybir.dt.float32
FP16 = mybir.dt.float16
BF16 = mybir.dt.bfloat16
EXP = mybir.ActivationFunctionType.Exp

B, T, HID, H, D = 2, 2048, 2048, 16, 128
NCORES = 8
CPB = 4              # cores per batch
HPC = H // CPB       # 4 heads per core
HD_PC = HPC * D      # 512 output dims per core
P = 128
KT = T // P          # 16 key tiles
NK = HID // P        # 16 contraction tiles for projections
NBLK = 4             # t blocks of 512
TPB = 4              # query tiles per block


def _rope_fold():
    """Per-head rotation matrices R_h (128x128) from the reference's quirky rope."""
    half = D // 2
    theta = 1.0 / (10000.0 ** (np.arange(0, half, 2, dtype=np.float64) / half))
    mats = []
    for h in range(H):
        R = np.zeros((D, D), dtype=np.float64)
        c = np.cos(h * theta)
        s = np.sin(h * theta)
        for j in range(32):
            R[j, 2 * j] = c[j]
            R[j, 2 * j + 1] = -s[j]
            R[32 + j, 2 * j] = s[j]
            R[32 + j, 2 * j + 1] = c[j]
            R[64 + j, 64 + 2 * j] = c[j]
            R[64 + j, 64 + 2 * j + 1] = -s[j]
            R[96 + j, 64 + 2 * j] = s[j]
            R[96 + j, 64 + 2 * j + 1] = c[j]
        mats.append(R)
    return mats


def _build_program():
    nc = bacc.Bacc("TRN2", target_bir_lowering=False, debug=False,
                   enable_asserts=False, num_devices=NCORES)

    hsT = nc.dram_tensor("hsT", [HID, T], FP16, kind="ExternalInput").ap()
    wqT = nc.dram_tensor("wqT", [HID, HD_PC], FP16, kind="ExternalInput").ap()
    wkvT = nc.dram_tensor("wkvT", [HID, 2 * D], FP16, kind="ExternalInput").ap()
    woT = nc.dram_tensor("woT", [HD_PC, HID], BF16, kind="ExternalInput").ap()
    dmd = nc.dram_tensor("dmask", [P, P], BF16, kind="ExternalInput").ap()
    onbf = nc.dram_tensor("onbf", [P, P], BF16, kind="ExternalInput").ap()
    seld = nc.dram_tensor("seld", [P, P], BF16, kind="ExternalInput").ap()
    out = nc.dram_tensor("out", [T, HID], BF16, kind="ExternalOutput").ap()

    hsT_r = hsT.rearrange("(ko p) t -> ko p t", p=P)        # [16,128,2048]
    wqT_r = wqT.rearrange("(ko p) m -> p ko m", p=P)        # [128,16,512]
    wkvT_r = wkvT.rearrange("(ko p) d -> p ko d", p=P)      # [128,16,256]
    woT_r = woT.rearrange("(h p) n -> p h n", p=P)          # [128,4,2048]
    out_r = out.rearrange("(tt p) n -> tt p n", p=P)        # [16,128,2048]

    def mm(ps, lhsT, rhs, start, stop):
        nc.tensor.matmul(ps, lhsT=lhsT, rhs=rhs, start=start, stop=stop)

    with tile.TileContext(nc) as tc, ExitStack() as ctx:
        singles = ctx.enter_context(tc.tile_pool(name="singles", bufs=1))
        hpool = ctx.enter_context(tc.tile_pool(name="hst", bufs=16))
        epool = ctx.enter_context(tc.tile_pool(name="etile", bufs=6))
        spool = ctx.enter_context(tc.tile_pool(name="small", bufs=2))
        apool = ctx.enter_context(tc.tile_pool(name="att", bufs=2))
        opool = ctx.enter_context(tc.tile_pool(name="outt", bufs=4))

        ident = singles.tile([P, P], F32)
        make_identity(nc, ident)
        ident_bf = singles.tile([P, P], BF16)
        nc.vector.tensor_copy(ident_bf[:], ident[:])
        dmask = singles.tile([P, P], BF16)
        ones_bf = singles.tile([P, P], BF16)
        sel_bf = singles.tile([P, P], BF16)
        nc.sync.dma_start(out=dmask, in_=dmd)
        nc.sync.dma_start(out=ones_bf, in_=onbf)
        nc.sync.dma_start(out=sel_bf, in_=seld)

        # weight residents; per-k slices stream in with the first block's
        # data. Wo rides the sync queue (outputs only start later).
        wq_sb = singles.tile([P, NK, HD_PC], FP16)
        wkv_sb = singles.tile([P, NK, 2 * D], FP16)
        wo_sb = singles.tile([P, HPC, HID], BF16)

        # resident activations
        qt_sb = singles.tile([P, HPC, T], FP16)      # Q^T per head [d, t]
        kt_sb = singles.tile([P, T], FP16)           # K^T [d, s]
        v_sb = singles.tile([P, KT, D], BF16)        # V natural [s-tile, d]

        dmask_b = dmask[:, None, :].to_broadcast([P, HPC, P])
        prev_at_ref = [None]  # at tile of the previous block (sweep deferral)

        def tail_a(pp, st_):
            """First half of a row's softmax tail: recombine strips, 1/den."""
            den_ps, strips = st_["den_ps"], st_["strips"]
            if strips:  # recombine the 4 column-tiled strip rows
                dstr = spool.tile([P, HPC, P], BF16, tag="dstr", name="dstr")
                nc.vector.tensor_copy(dstr[:], den_ps[:])
                dt_ps = pp.tile([P, HPC, P], F32, tag="pp", name="dt_ps")
                mm(dt_ps[:1], sel_bf[:, :1], dstr[:], True, True)
                den_ap = dt_ps[:1]
            else:
                den_ap = den_ps[:1]
            recip = spool.tile([1, HPC, P], F32, tag="recip", name="recip")
            nc.vector.reciprocal_approx_fast(out=recip[:], in_=den_ap)
            recr = spool.tile([1, HPC, P], BF16, tag="recr", name="recr")
            nc.vector.tensor_copy(recr[:], recip[:])
            st_["recr"] = recr

        def tail_b(pp, st_):
            """Second half: broadcast 1/den across partitions, normalize O^T."""
            bc_ps = pp.tile([P, HPC, P], F32, tag="pp", name="bc_ps")
            mm(bc_ps[:], ones_bf[:1, :], st_["recr"][:], True, True)
            bc_sb = spool.tile([P, HPC, P], F32, tag="bc_sb", name="bc_sb")
            nc.vector.tensor_copy(bc_sb[:], bc_ps[:])
            nc.vector.tensor_mul(st_["at_blk"][:, :, st_["tt"], :],
                                 st_["ot_ps"][:], bc_sb[:])

        for blk in range(NBLK):
            tsl = slice(blk * 512, (blk + 1) * 512)
            # ---------------- projection for this 512-block ----------------
            with tc.tile_pool(name="ps1", bufs=1, space="PSUM") as ps1:
                q_ps = [ps1.tile([P, 512], F32, tag=f"qps{h}", name=f"qps{h}")
                        for h in range(HPC)]
                k_ps = ps1.tile([P, 512], F32, tag="kps")
                v_ps = ps1.tile([P, 512], F32, tag="vps")
                for k in range(NK):
                    if blk == 0:  # stream weight slices just ahead of data
                        nc.sync.dma_start(out=wkv_sb[:, k, :], in_=wkvT_r[:, k, :])
                        nc.scalar.dma_start(out=wq_sb[:, k, :], in_=wqT_r[:, k, :])
                    hst = hpool.tile([P, 512], FP16)
                    nc.gpsimd.dma_start(out=hst, in_=hsT_r[k][:, tsl])
                    st, sp = (k == 0), (k == NK - 1)
                    for h in range(HPC):
                        mm(q_ps[h][:], wq_sb[:, k, h * D:(h + 1) * D], hst[:], st, sp)
                    mm(k_ps[:], wkv_sb[:, k, :D], hst[:], st, sp)
                    mm(v_ps[:], wkv_sb[:, k, D:], hst[:], st, sp)
                if blk == 0:  # out-proj weights: first needed at block 0's sweep
                    for h in range(HPC):
                        for jb in range(4):
                            nc.sync.dma_start(
                                out=wo_sb[:, h, jb * 512:(jb + 1) * 512],
                                in_=woT_r[:, h, jb * 512:(jb + 1) * 512])
                # vt copy first so the V transposes (emitted in the rows
                # section) can start early; qt in row-tile slices so row
                # blk*4's scores start after 4 small copies.
                vt_sb = spool.tile([P, 512], BF16, tag="vt")
                nc.vector.tensor_copy(vt_sb[:], v_ps[:])
                for tt in range(TPB):
                    for h in range(HPC):
                        qsl = slice(blk * 512 + tt * P, blk * 512 + (tt + 1) * P)
                        nc.vector.tensor_copy(qt_sb[:, h, qsl],
                                              q_ps[h][:, tt * P:(tt + 1) * P])
                nc.vector.tensor_copy(kt_sb[:, tsl], k_ps[:])

            # ---------------- attention rows of this block -----------------
            # Software-pipelined pair stream: scores+exp for pair i+1 are
            # emitted before AV/den of pair i, so the PE never sits behind
            # an AV that waits on the ACT exp chain. Row tails (two PE ops +
            # DVE chain) advance one stage per pair, ahead of the AVs that
            # would deadlock on them.
            at_blk = apool.tile([P, HPC, TPB, P], BF16, tag="atb", name="at_blk")
            pairs = []  # (tt, st0, npair)
            for tt in range(TPB):
                tb = blk * TPB + tt
                for st0 in range(0, tb + 1, 2):
                    pairs.append((tt, st0, min(2, tb + 1 - st0)))
            rowstate = {}
            tails = []  # pend dicts: tail_a then tail_b, one stage per pair

            with tc.tile_pool(name="ps2s", bufs=2, space="PSUM") as ps2s, \
                 tc.tile_pool(name="ps2o", bufs=2, space="PSUM") as ps2o, \
                 tc.tile_pool(name="ps2d", bufs=1, space="PSUM") as ps2d, \
                 tc.tile_pool(name="ps2b", bufs=1, space="PSUM") as ps2b:

                def emit_scores(pr):
                    tt, st0, npair = pr
                    tb = blk * TPB + tt
                    qrhs = qt_sb[:, :, tb * P:(tb + 1) * P]
                    s_ps = ps2s.tile([P, 2, HPC, P], F32, tag="sps", name="s_ps")
                    for i in range(npair):
                        st = st0 + i
                        mm(s_ps[:, i], kt_sb[:, st * P:(st + 1) * P],
                           qrhs, True, True)
                    e_sb = epool.tile([P, 2, HPC, P], BF16, tag="etile",
                                      name="e_sb")
                    if npair == 2:
                        nc.scalar.activation(e_sb[:], s_ps[:], EXP)
                    else:
                        nc.scalar.activation(e_sb[:, 0], s_ps[:, 0], EXP)
                    if st0 + npair - 1 == tb:  # diagonal tile: causal mask
                        nc.vector.tensor_mul(e_sb[:, npair - 1],
                                             e_sb[:, npair - 1], dmask_b)
                    return e_sb

                def emit_tail_stage():
                    if not tails:
                        return
                    st_ = tails[0]
                    if st_["stage"] == 0:
                        tail_a(ps2b, st_)
                        st_["stage"] = 1
                    else:
                        tail_b(ps2b, st_)
                        tails.pop(0)

                def emit_avden(pr, e_sb):
                    tt, st0, npair = pr
                    tb = blk * TPB + tt
                    strips = tb >= 3
                    if tt not in rowstate:
                        rowstate[tt] = {
                            "ot_ps": ps2o.tile([P, HPC, P], F32, tag="ot",
                                               name="ot_ps"),
                            "den_ps": ps2d.tile([P, HPC, P], F32, tag="den",
                                                name="den_ps"),
                            "strips": strips, "pstrips": [],
                            "at_blk": at_blk, "tt": tt, "stage": 0,
                        }
                    rs = rowstate[tt]
                    for i in range(npair):
                        st = st0 + i
                        if strips:
                            rs["pstrips"].append((st, e_sb[:, i]))
                        else:
                            mm(rs["den_ps"][:1], ones_bf[:, :1], e_sb[:, i],
                               st == 0, st == tb)
                        mm(rs["ot_ps"][:], v_sb[:, st, :], e_sb[:, i],
                           st == 0, st == tb)
                    # den strips: back-to-back groups of 4 run concurrently
                    # in distinct PE column groups
                    if len(rs["pstrips"]) >= 4 or st0 + npair - 1 == tb:
                        for st, e_ap in rs["pstrips"]:
                            j = st % 4
                            nc.tensor.matmul(
                                rs["den_ps"][32 * j:32 * j + 1, :],
                                lhsT=ones_bf[:, :1], rhs=e_ap,
                                start=(st < 4), stop=(st + 4 > tb),
                                tile_position=(0, 32 * j))
                        rs["pstrips"] = []
                    if st0 + npair - 1 == tb:  # row complete -> queue tail
                        tails.append(rs)

                # V^T -> V natural via PE transpose (bf16), interleaved
                # with the first scores so the block transition has no
                # PE bubble; AVs need this block's v only at its diagonal.
                e_q = [emit_scores(pairs[0])]
                for si in range(4):
                    pt = ps2b.tile([P, HPC, P], BF16, tag="pp", name="pt")
                    nc.tensor.transpose(pt[:, 0, :],
                                        vt_sb[:, si * P:(si + 1) * P],
                                        ident_bf[:])
                    nc.vector.tensor_copy(v_sb[:, blk * 4 + si, :],
                                          pt[:, 0, :])
                if len(pairs) > 1:
                    e_q.append(emit_scores(pairs[1]))
                for i, pr in enumerate(pairs):
                    if i + 2 < len(pairs):
                        e_q.append(emit_scores(pairs[i + 2]))
                    emit_tail_stage()
                    emit_avden(pr, e_q.pop(0))

                # ---------- out-projection sweep (same pool scope) ----------
                # op tiles reuse the scores slots; the last rows' leftover
                # tail stages interleave between the first op groups (sweep
                # tiles tt<=1 don't depend on them). Output DMAs go out in
                # 512-col quarters so a 512KB transfer never head-of-line
                # blocks an hs prefetch on its DMA engine.
                for tt in range(TPB):
                    tb = blk * TPB + tt
                    oto = opool.tile([P, 2048], BF16, tag="oto")
                    for jb2 in range(2):
                        op_ps = ps2s.tile([P, 2, 512], F32, tag="sps",
                                          name="op_ps")
                        for half in range(2):
                            jsl = slice((2 * jb2 + half) * 512,
                                        (2 * jb2 + half + 1) * 512)
                            for h in range(HPC):
                                mm(op_ps[:, half], at_blk[:, h, tt, :],
                                   wo_sb[:, h, jsl], h == 0, h == HPC - 1)
                        emit_tail_stage()
                        nc.vector.tensor_copy(
                            oto[:, jb2 * 1024:(jb2 + 1) * 1024], op_ps[:])
                    for q in range(4):
                        qs = slice(q * 512, (q + 1) * 512)
                        nc.sync.dma_start(out=out_r[tb][:, qs], in_=oto[:, qs])

    nc.compile()
    return nc


_CACHE = {}


def _get_program():
    if "nc" not in _CACHE:
        _CACHE["nc"] = _build_program()
    return _CACHE["nc"]


def _host_inputs(hidden_states, Wq, Wk, Wv, Wo):
    """Fold rope+scale into weights, build per-core input maps."""
    f64 = np.float64
    mats = _rope_fold()
    scale = D ** -0.5
    Wq_f = np.empty((HID, HID), dtype=np.float32)
    for h in range(H):
        Wq_f[h * D:(h + 1) * D] = (mats[h] @ Wq[h * D:(h + 1) * D].astype(f64)
                                   * scale).astype(np.float32)
    perm = np.concatenate([np.arange(0, 64, 2), np.arange(1, 64, 2),
                           np.arange(64, 128, 2), np.arange(65, 128, 2)])
    Wk_f = Wk[perm].astype(np.float32)

    wkvT = np.ascontiguousarray(
        np.concatenate([Wk_f.T, Wv.T], axis=1)).astype(np.float16)
    ii = np.arange(P)[:, None]
    jj = np.arange(P)[None, :]
    dmask = (ii <= jj).astype(ml_dtypes.bfloat16)
    sel = np.zeros((P, P), dtype=np.float32)
    sel[[0, 32, 64, 96], 0] = 1.0

    hsT = [np.ascontiguousarray(hidden_states[b].T).astype(np.float16)
           for b in range(B)]
    in_maps = []
    for c in range(NCORES):
        b, q = c // CPB, c % CPB
        rows = slice(q * HD_PC, (q + 1) * HD_PC)
        in_maps.append({
            "hsT": hsT[b],
            "wqT": np.ascontiguousarray(Wq_f[rows].T).astype(np.float16),
            "wkvT": wkvT,
            "woT": np.ascontiguousarray(Wo[:, rows].T).astype(ml_dtypes.bfloat16),
            "dmask": dmask,
            "onbf": np.ones((P, P), dtype=ml_dtypes.bfloat16),
            "seld": sel.astype(ml_dtypes.bfloat16),
        })
    return in_maps


def kernel(hidden_states, Wq, Wk, Wv, Wo):
    hidden_states = np.asarray(hidden_states, dtype=np.float32)
    Wq = np.asarray(Wq, dtype=np.float32)
    Wk = np.asarray(Wk, dtype=np.float32)
    Wv = np.asarray(Wv, dtype=np.float32)
    Wo = np.asarray(Wo, dtype=np.float32)

    nc = _get_program()
    in_maps = _host_inputs(hidden_states, Wq, Wk, Wv, Wo)
    res = run_bass_kernel_spmd(nc, in_maps, list(range(NCORES)))
    parts = [np.asarray(r["out"]).astype(np.float32) for r in res.results]
    out = np.empty((B, T, HID), dtype=np.float32)
    for b in range(B):
        out[b] = parts[CPB * b]
        for q in range(1, CPB):
            out[b] += parts[CPB * b + q]
    return out
